# revision 1
# baseline (speedup 1.0000x reference)
"""Trainium2 Bass kernel for nn_ContrastiveDist (supervised contrastive loss).

Math
----
The reference builds (n,n) distance/weight matrices, but the loss collapses
exactly to per-class statistics.  With classes c = 0..15, per-class count
cnt[c], feature sums C[c,:], squared-norm sums SqSum[c], global sums
Ftot / SSall:

    alpha[c] = 1/(cnt[c]-1+eps)
    beta[c]  = 1/(n-cnt[c]+eps)
    loss_i   = sq_i*P[c_i] + (Q[c_i]+M) + f_i . R[c_i]
      P[c]   = alpha*cnt - beta*(n-cnt)
      Q[c]   = alpha*SqSum[c] - beta*(SSall-SqSum[c])
      R[c,:] = 2*beta*(Ftot-C[c]) - 2*alpha*C[c]
    result   = sum(relu(loss_i)*valid_i) / max(sum(valid_i), 1)

valid_i = (cnt[c_i] >= 2) is folded into the coefficients: Raug rows of
invalid classes are zeroed, so relu(loss) = 0 there, and the valid count
comes from sum(cnt[c]*vmask[c]).  Validated to ~4e-7 rel vs the f32
reference (sim).

Distribution: every core redundantly computes the full loss (inputs are
replicated).  No collectives: a cross-core AllGather costs ~9us plus a
~45us rank-skew barrier under this dispatch path, far more than the
~15us of redundant per-core compute it would save.
"""

import numpy as np
import ml_dtypes

import concourse.bacc as bacc
import concourse.tile as tile
import concourse.mybir as mybir
from concourse.bass_utils import run_bass_kernel_spmd

N, D, K, NCORES = 8192, 128, 16, 8
T = N // 128               # 64 row-tiles of 128
W = D + 3                  # faug stride: [F(128), sq, 1, pad]
EPS, MARGIN = 1e-6, 10.0
F32 = mybir.dt.float32
BF16 = mybir.dt.bfloat16
Alu = mybir.AluOpType
Act = mybir.ActivationFunctionType
AxX = mybir.AxisListType.X

# const tensor (128, CW) f32:
#   cols 0:16   iota c (one-hot compare operand, broadcast over tiles)
#   col 16      1.0  (ones(128,1) lhsT for the final partition reduce)
#   cols 17:33  1.0  (rows 0:16 = ones(16,16) lhsT for global-sum broadcast)
CW = 34

_CACHE: dict = {}


def _build():
    if "nc" in _CACHE:
        return _CACHE["nc"]

    nc = bacc.Bacc("TRN2", target_bir_lowering=False, debug=False, num_devices=NCORES)
    fain = nc.dram_tensor("fain", [128, T * W], F32, kind="ExternalInput").ap()
    fhin = nc.dram_tensor("fhin", [128, T * W], BF16, kind="ExternalInput").ap()
    flin = nc.dram_tensor("flin", [128, T * W], BF16, kind="ExternalInput").ap()
    labrep = nc.dram_tensor("labrep", [128, T * 16], F32, kind="ExternalInput").ap()
    lab16 = nc.dram_tensor("lab16", [16, N], BF16, kind="ExternalInput").ap()
    c16b = nc.dram_tensor("c16b", [16, 1], F32, kind="ExternalInput").ap()
    cst = nc.dram_tensor("cst", [128, CW], F32, kind="ExternalInput").ap()
    res = nc.dram_tensor("res", [1, 1], F32, kind="ExternalOutput").ap()

    with tile.TileContext(nc) as tc:
        with (
            tc.tile_pool(name="sb", bufs=1) as sb,
            tc.tile_pool(name="ps", bufs=1, space="PSUM") as ps,
        ):
            # ---------------- loads ----------------
            csts = sb.tile([128, CW], F32)
            nc.sync.dma_start(csts[:], cst)
            labs = sb.tile([128, T * 16], F32)
            nc.gpsimd.dma_start(labs[:], labrep)
            lab16s = sb.tile([16, N], BF16)
            nc.gpsimd.dma_start(lab16s[:], lab16)
            c16s = sb.tile([16, 1], F32)
            nc.gpsimd.dma_start(c16s[:], c16b)

            faug = sb.tile([128, T * W], F32)
            fa3 = faug.rearrange("p (t w) -> p t w", w=W)
            faugh = sb.tile([128, W * T], BF16)
            fh3 = faugh.rearrange("p (w t) -> p w t", t=T)
            faugl = sb.tile([128, W * T], BF16)
            fl3 = faugl.rearrange("p (w t) -> p w t", t=T)
            CH = T * W // 4
            for g in range(4):  # contiguous 2D chunks, alternate queues
                eng = nc.sync if g % 2 == 0 else nc.gpsimd
                eng.dma_start(faug[:, g * CH:(g + 1) * CH],
                              fain[:, g * CH:(g + 1) * CH])
            for g in range(2):
                nc.sync.dma_start(faugh[:, g * 2 * CH:(g + 1) * 2 * CH],
                                  fhin[:, g * 2 * CH:(g + 1) * 2 * CH])
                nc.gpsimd.dma_start(faugl[:, g * 2 * CH:(g + 1) * 2 * CH],
                                    flin[:, g * 2 * CH:(g + 1) * 2 * CH])

            # ---------------- one-hots ----------------
            eohaf = sb.tile([128, T * 16], F32)
            eohf3 = eohaf.rearrange("p (t c) -> p t c", c=16)
            iota3 = csts[:, 0:16].unsqueeze(1).broadcast_to((128, T, 16))
            lab3 = labs.rearrange("p (t c) -> p t c", c=16)
            nc.vector.tensor_tensor(eohf3[:, :, :], iota3, lab3, op=Alu.is_equal)
            eoha = sb.tile([128, T * 16], BF16)
            eoh3 = eoha.rearrange("p (t c) -> p t c", c=16)
            nc.vector.tensor_copy(eoha[:], eohaf[:])
            eohT = sb.tile([16, N], BF16)
            nc.vector.tensor_scalar(eohT[:], lab16s[:], c16s[:], None,
                                    op0=Alu.is_equal)

            # ---------------- sq_i then per-class stats ----------------
            ffbig = sb.tile([128, T * D], F32)
            ff3 = ffbig.rearrange("p (t d) -> p t d", d=D)
            nc.scalar.activation(ff3[:, :, :], fa3[:, :, 0:D], Act.Square)
            sqd = sb.tile([128, T], F32)
            nc.vector.tensor_reduce(sqd[:], ff3, axis=AxX, op=Alu.add)
            nc.vector.tensor_copy(fa3[:, :, D], sqd[:])
            nc.vector.tensor_copy(fh3[:, D, :], sqd[:])
            nc.vector.scalar_tensor_tensor(fl3[:, D, :], sqd[:], 0.0,
                                           fh3[:, D, :],
                                           op0=Alu.bypass, op1=Alu.subtract)

            statsP = ps.tile([16, D + 2], F32)
            for t in range(T):
                nc.tensor.matmul(statsP[:], eoh3[:, t, :], fh3[:, 0:D + 2, t],
                                 start=(t == 0), stop=False)
                nc.tensor.matmul(statsP[:], eoh3[:, t, :], fl3[:, 0:D + 2, t],
                                 start=False, stop=(t == T - 1))
            stats = sb.tile([16, D + 2], F32)
            nc.vector.tensor_copy(stats[:], statsP[:])

            # ---------------- per-class coefficients ----------------
            C = stats[:, 0:D]
            SqS = stats[:, D:D + 1]
            cnt = stats[:, D + 1:D + 2]
            gbP = ps.tile([16, D + 2], F32)
            nc.tensor.matmul(gbP[:], csts[0:16, 17:33], stats[:],
                             start=True, stop=True)
            gb = sb.tile([16, D + 2], F32)
            nc.vector.tensor_copy(gb[:], gbP[:])
            Ftot = gb[:, 0:D]
            SSall = gb[:, D:D + 1]

            alpha = sb.tile([16, 1], F32)
            nc.vector.tensor_scalar(alpha[:], cnt, EPS - 1.0, None, op0=Alu.add)
            nc.vector.reciprocal(alpha[:], alpha[:])
            beta = sb.tile([16, 1], F32)
            nc.vector.tensor_scalar(beta[:], cnt, -1.0, float(N) + EPS,
                                    op0=Alu.mult, op1=Alu.add)
            nc.vector.reciprocal(beta[:], beta[:])
            nalpha2 = sb.tile([16, 1], F32)
            nc.vector.tensor_scalar(nalpha2[:], alpha[:], -2.0, None, op0=Alu.mult)
            beta2 = sb.tile([16, 1], F32)
            nc.vector.tensor_scalar(beta2[:], beta[:], 2.0, None, op0=Alu.mult)

            raug = sb.tile([16, D + 2], F32)
            tmpd = sb.tile([16, D], F32)
            nc.vector.tensor_tensor(tmpd[:], Ftot, C, op=Alu.subtract)
            nc.vector.tensor_scalar(tmpd[:], tmpd[:], beta2[:], None, op0=Alu.mult)
            nc.vector.scalar_tensor_tensor(raug[:, 0:D], C, nalpha2[:], tmpd[:],
                                           op0=Alu.mult, op1=Alu.add)
            nmc = sb.tile([16, 1], F32)
            nc.vector.tensor_scalar(nmc[:], cnt, -1.0, float(N),
                                    op0=Alu.mult, op1=Alu.add)
            nc.vector.tensor_tensor(nmc[:], nmc[:], beta[:], op=Alu.mult)
            nc.vector.scalar_tensor_tensor(raug[:, D:D + 1], cnt, alpha[:], nmc[:],
                                           op0=Alu.mult, op1=Alu.subtract)
            ssd = sb.tile([16, 1], F32)
            nc.vector.tensor_tensor(ssd[:], SSall, SqS, op=Alu.subtract)
            nc.vector.tensor_tensor(ssd[:], ssd[:], beta[:], op=Alu.mult)
            qa = sb.tile([16, 1], F32)
            nc.vector.scalar_tensor_tensor(qa[:], SqS, alpha[:], ssd[:],
                                           op0=Alu.mult, op1=Alu.subtract)
            nc.vector.tensor_scalar(raug[:, D + 1:D + 2], qa[:], MARGIN, None,
                                    op0=Alu.add)

            # fold validity into the coefficients: zero Raug rows of classes
            # with cnt < 2, so relu(loss) vanishes for invalid rows
            vmask = sb.tile([16, 1], F32)
            nc.vector.tensor_scalar(vmask[:], cnt, 1.5, None, op0=Alu.is_ge)
            nc.vector.tensor_scalar(raug[:], raug[:], vmask[:], None, op0=Alu.mult)

            # bf16 hi/lo split of raug -> two-chain bf16 matmul ~= fp32 exact
            rhi = sb.tile([16, D + 2], BF16)
            nc.vector.tensor_copy(rhi[:], raug[:])
            rlo32 = sb.tile([16, D + 2], F32)
            nc.vector.tensor_tensor(rlo32[:], raug[:], rhi[:], op=Alu.subtract)
            rlo = sb.tile([16, D + 2], BF16)
            nc.vector.tensor_copy(rlo[:], rlo32[:])

            # ---------------- per-row losses ----------------
            lossrows = sb.tile([128, T], F32)
            for r in range(T // 2):  # 32 rounds x 2 tiles; D-psum 2 banks x2 slots
                dP = ps.tile([128, 2 * 512], F32, tag="dpsum", bufs=2,
                             name=f"dP{r}")
                d3 = dP.rearrange("p (b x) -> p b x", x=512)
                for j in range(2):
                    t = r * 2 + j
                    lhs = eohT[:, t * 128:(t + 1) * 128]
                    nc.tensor.matmul(d3[:, j, 0:D + 2], lhs, rhi[:],
                                     start=True, stop=False)
                    nc.tensor.matmul(d3[:, j, 0:D + 2], lhs, rlo[:],
                                     start=False, stop=True)
                for j in range(2):
                    t = r * 2 + j
                    pscr = sb.tile([128, D + 2], F32, tag="pscr", bufs=4,
                                   name=f"ps{r}_{j}")
                    nc.vector.scalar_tensor_tensor(
                        pscr[:], d3[:, j, 0:D + 2], 0.0, fa3[:, t, 0:D + 2],
                        op0=Alu.bypass, op1=Alu.mult,
                        accum_out=lossrows[:, t:t + 1])

            # ---------------- final reduction ----------------
            accpair = sb.tile([128, 2], F32)
            nc.gpsimd.memset(accpair[:, 1:2], 0.0)
            relscr = sb.tile([128, T], F32)
            nc.vector.tensor_scalar(relscr[:], lossrows[:], 0.0, None,
                                    op0=Alu.max, op1=Alu.add,
                                    accum_out=accpair[:, 0:1])
            nc.vector.tensor_tensor(accpair[0:16, 1:2], cnt, vmask[:],
                                    op=Alu.mult)
            finP = ps.tile([1, 2], F32)
            nc.tensor.matmul(finP[:], csts[:, 16:17], accpair[:],
                             start=True, stop=True)
            fin = sb.tile([1, 2], F32)
            nc.vector.tensor_copy(fin[:], finP[:])
            den = sb.tile([1, 1], F32)
            nc.vector.tensor_scalar(den[:], fin[:, 1:2], 1.0, None, op0=Alu.max)
            nc.vector.reciprocal(den[:], den[:])
            resS = sb.tile([1, 1], F32)
            nc.vector.tensor_tensor(resS[:], fin[:, 0:1], den[:], op=Alu.mult)
            nc.sync.dma_start(res, resS[:])

    nc.compile()
    _CACHE["nc"] = nc
    return nc


def _make_in_maps(features, labels):
    feats = np.ascontiguousarray(np.asarray(features, dtype=np.float32))
    lab = np.ascontiguousarray(np.asarray(labels)).astype(np.float32)

    cst = np.zeros((128, CW), np.float32)
    cst[:, 0:16] = np.arange(16, dtype=np.float32)[None, :]
    cst[:, 16:33] = 1.0

    fa = np.zeros((128, T, W), np.float32)
    fa[:, :, 0:D] = feats.reshape(T, 128, D).transpose(1, 0, 2)
    fa[:, :, D + 1] = 1.0
    fawt = np.ascontiguousarray(fa.transpose(0, 2, 1))  # (128, W, T)
    fh = fawt.reshape(128, W * T).astype(ml_dtypes.bfloat16)
    fl = (fawt.reshape(128, W * T) - fh.astype(np.float32)).astype(
        ml_dtypes.bfloat16)
    fa = fa.reshape(128, T * W)

    one = {
        "fain": fa,
        "fhin": fh,
        "flin": fl,
        "labrep": np.ascontiguousarray(
            np.repeat(lab.reshape(T, 128).T, 16, axis=1)),
        "lab16": np.ascontiguousarray(
            np.broadcast_to(lab, (16, N))).astype(ml_dtypes.bfloat16),
        "c16b": np.arange(16, dtype=np.float32).reshape(16, 1),
        "cst": cst,
    }
    return [dict(one) for _ in range(NCORES)]


def kernel(features, labels):
    nc = _build()
    in_maps = _make_in_maps(features, labels)
    out = run_bass_kernel_spmd(nc, in_maps, core_ids=list(range(NCORES)))
    return np.float32(out.results[0]["res"][0, 0])



# revision 4
# speedup vs baseline: 2.5305x; 2.5305x over previous
"""Trainium2 Bass kernel for nn_ContrastiveDist (supervised contrastive loss).

Math (identical collapse to v1 baseline)
----------------------------------------
The (n,n) distance/weight loss collapses to per-class statistics.  With
classes c = 0..15, class sizes cnt[c], feature sums C[c,:], squared-norm
sums SqS[c], global sums Ftot / SSall:

    alpha[c] = 1/(cnt[c]-1+eps),  beta[c] = 1/(n-cnt[c]+eps)
    P[c]   = alpha*cnt - beta*(n-cnt)
    Q[c]   = alpha*SqS - beta*(SSall-SqS)
    R[c,:] = 2*beta*(Ftot-C) - 2*alpha*C
    loss_i = f_i . R[c_i] + sq_i*P[c_i] + Q[c_i] + M
    result = sum(relu(loss_i)*valid_i) / max(sum(valid_i), 1)

v2 changes vs the 110us baseline:
  * single bf16 matmul chain (no hi/lo split).  M=10 kept in its own
    bf16-exact Raug column so only Q (~0.25) sees bf16 rounding; numpy
    sim of this exact quantization chain gives rel err 3.9e-5.
  * per-row loss phase is sharded: core k computes row-tiles [8k, 8k+8);
    per-core partial sums are combined on the host (no collectives).
  * features travel once, in bf16 (2.1 MB) + an exact f32 slice of this
    core's 8 tiles for the final dot; total DMA ~2.9 MB vs 9.4 MB.
  * squares/reduce (for sq_i) chunked and pipelined under the DMA;
    stats matmul chain follows chunk arrival.
  * few large dma_starts (SWDGE fixed cost ~1us per dma_start).
"""

import numpy as np
import ml_dtypes

import concourse.bacc as bacc
import concourse.tile as tile
import concourse.mybir as mybir
from concourse.bass_utils import run_bass_kernel_spmd

N, D, K, NCORES = 8192, 128, 16, 8
T = N // 128               # 64 row-tiles of 128
TC = T // NCORES           # 8 row-tiles per core for the loss phase
W = D + 2                  # fh cols: [f(128) | sq | 1]
W3 = D + 3                 # fa32/Raug cols: [f(128) | sq | 1 | 1] / [R|P|Q|Mvm]
NCH = 4                    # fh DMA chunks
CHT = T // NCH             # tiles per chunk
EPS, MARGIN = 1e-6, 10.0
F32 = mybir.dt.float32
BF16 = mybir.dt.bfloat16
Alu = mybir.AluOpType
Act = mybir.ActivationFunctionType
AxX = mybir.AxisListType.X

_CACHE: dict = {}


def _build():
    if "nc" in _CACHE:
        return _CACHE["nc"]

    nc = bacc.Bacc("TRN2", target_bir_lowering=False, debug=False, num_devices=NCORES)
    # fhin: bf16 row-tile-major features [f | sq_slot(0) | 1] per tile
    fhin = nc.dram_tensor("fhin", [128, T * W], BF16, kind="ExternalInput").ap()
    # labio: labels replicated x16 (cols 0:T*16) ++ iota row (cols T*16:T*16+16)
    labio = nc.dram_tensor("labio", [128, T * 16 + 16], BF16,
                           kind="ExternalInput").ap()
    # lab16o: this core's 1024 labels broadcast to 16 partitions
    lab16o = nc.dram_tensor("lab16o", [16, TC * 128], BF16,
                            kind="ExternalInput").ap()
    # smallf: col 0 = iota16, cols 1:17 = ones(16,16)
    smallf = nc.dram_tensor("smallf", [16, 17], F32, kind="ExternalInput").ap()
    # fa32in: this core's 8 tiles in f32: [f(128) | 0 | 1 | 1]
    fa32in = nc.dram_tensor("fa32in", [128, TC * W3], F32,
                            kind="ExternalInput").ap()
    res = nc.dram_tensor("res", [128, 2], F32, kind="ExternalOutput").ap()

    with tile.TileContext(nc) as tc:
        with (
            tc.tile_pool(name="sb", bufs=1) as sb,
            tc.tile_pool(name="ps", bufs=1, space="PSUM") as ps,
        ):
            # ---------------- loads ----------------
            labs = sb.tile([128, T * 16 + 16], BF16)
            nc.sync.dma_start(labs[:], labio)
            smf = sb.tile([16, 17], F32)
            nc.gpsimd.dma_start(smf[:], smallf)
            lab16s = sb.tile([16, TC * 128], BF16)
            nc.gpsimd.dma_start(lab16s[:], lab16o)

            fh = sb.tile([128, T * W], BF16)
            fh3 = fh.rearrange("p (t w) -> p t w", w=W)
            CHW = CHT * W
            for g in range(NCH):  # alternate queues; arrival order ~ 0,1,2,3
                eng = nc.sync if g % 2 == 0 else nc.gpsimd
                eng.dma_start(fh[:, g * CHW:(g + 1) * CHW],
                              fhin[:, g * CHW:(g + 1) * CHW])
            fa32 = sb.tile([128, TC * W3], F32)
            fa3 = fa32.rearrange("p (t w) -> p t w", w=W3)
            nc.gpsimd.dma_start(fa32[:], fa32in)

            # ---------------- one-hots (bf16) ----------------
            eoh = sb.tile([128, T * 16], BF16)
            eoh3 = eoh.rearrange("p (t c) -> p t c", c=16)
            iota3 = labs[:, T * 16:T * 16 + 16].unsqueeze(1).broadcast_to(
                (128, T, 16))
            lab3 = labs[:, 0:T * 16].rearrange("p (t c) -> p t c", c=16)
            nc.vector.tensor_tensor(eoh3[:, :, :], iota3, lab3, op=Alu.is_equal)
            eohT = sb.tile([16, TC * 128], BF16)
            nc.vector.tensor_scalar(eohT[:], lab16s[:], smf[:, 0:1], None,
                                    op0=Alu.is_equal)

            # ---------------- sq_i pipeline (per chunk) ----------------
            sqd = sb.tile([128, T], F32)
            for g in range(NCH):
                scr = sb.tile([128, CHT * D], F32, tag="sqscr", bufs=2,
                              name=f"sq{g}")
                s3 = scr.rearrange("p (t d) -> p t d", d=D)
                nc.scalar.activation(s3[:, :, :],
                                     fh3[:, g * CHT:(g + 1) * CHT, 0:D],
                                     Act.Square)
                nc.vector.tensor_reduce(sqd[:, g * CHT:(g + 1) * CHT],
                                        s3, axis=AxX, op=Alu.add)
                # bf16 sq into fh col D (strided)
                nc.vector.tensor_copy(fh3[:, g * CHT:(g + 1) * CHT, D],
                                      sqd[:, g * CHT:(g + 1) * CHT])

            # ---------------- per-class stats (single bf16 chain) ----------
            statsP = ps.tile([16, W], F32)
            for t in range(T):
                nc.tensor.matmul(statsP[:], eoh3[:, t, :], fh3[:, t, 0:W],
                                 start=(t == 0), stop=(t == T - 1))
            stats = sb.tile([16, W], F32)
            nc.vector.tensor_copy(stats[:], statsP[:])

            # ---------------- per-class coefficients ----------------
            C = stats[:, 0:D]
            SqS = stats[:, D:D + 1]
            cnt = stats[:, D + 1:D + 2]
            gbP = ps.tile([16, W], F32)
            nc.tensor.matmul(gbP[:], smf[:, 1:17], stats[:],
                             start=True, stop=True)
            gb = sb.tile([16, W], F32)
            nc.vector.tensor_copy(gb[:], gbP[:])
            Ftot = gb[:, 0:D]
            SSall = gb[:, D:D + 1]

            alpha = sb.tile([16, 1], F32)
            nc.vector.tensor_scalar(alpha[:], cnt, EPS - 1.0, None, op0=Alu.add)
            nc.vector.reciprocal(alpha[:], alpha[:])
            beta = sb.tile([16, 1], F32)
            nc.vector.tensor_scalar(beta[:], cnt, -1.0, float(N) + EPS,
                                    op0=Alu.mult, op1=Alu.add)
            nc.vector.reciprocal(beta[:], beta[:])
            nab2 = sb.tile([16, 1], F32)   # -2*(alpha+beta)
            nc.vector.scalar_tensor_tensor(nab2[:], alpha[:], 1.0, beta[:],
                                           op0=Alu.bypass, op1=Alu.add)
            nc.vector.tensor_scalar(nab2[:], nab2[:], -2.0, None, op0=Alu.mult)
            beta2 = sb.tile([16, 1], F32)
            nc.vector.tensor_scalar(beta2[:], beta[:], 2.0, None, op0=Alu.mult)

            raug = sb.tile([16, W3], F32)
            nc.gpsimd.memset(raug[:, D + 2:D + 3], MARGIN)
            tmpd = sb.tile([16, D], F32)
            # R = C*(-2a-2b) + Ftot*2b
            nc.vector.tensor_scalar(tmpd[:], Ftot, beta2[:], None, op0=Alu.mult)
            nc.vector.scalar_tensor_tensor(raug[:, 0:D], C, nab2[:], tmpd[:],
                                           op0=Alu.mult, op1=Alu.add)
            # P = a*cnt - b*(N-cnt)
            nmc = sb.tile([16, 1], F32)
            nc.vector.tensor_scalar(nmc[:], cnt, -1.0, float(N),
                                    op0=Alu.mult, op1=Alu.add)
            nc.vector.tensor_tensor(nmc[:], nmc[:], beta[:], op=Alu.mult)
            nc.vector.scalar_tensor_tensor(raug[:, D:D + 1], cnt, alpha[:],
                                           nmc[:], op0=Alu.mult,
                                           op1=Alu.subtract)
            # Q = a*SqS - b*(SSall-SqS)
            ssd = sb.tile([16, 1], F32)
            nc.vector.tensor_tensor(ssd[:], SSall, SqS, op=Alu.subtract)
            nc.vector.tensor_tensor(ssd[:], ssd[:], beta[:], op=Alu.mult)
            nc.vector.scalar_tensor_tensor(raug[:, D + 1:D + 2], SqS, alpha[:],
                                           ssd[:], op0=Alu.mult,
                                           op1=Alu.subtract)
            # validity fold: zero rows of classes with cnt < 2
            vmask = sb.tile([16, 1], F32)
            nc.vector.tensor_scalar(vmask[:], cnt, 1.5, None, op0=Alu.is_ge)
            nc.vector.tensor_scalar(raug[:], raug[:], vmask[:], None,
                                    op0=Alu.mult)
            rb = sb.tile([16, W3], BF16)
            nc.vector.tensor_copy(rb[:], raug[:])

            # ---------------- this core's row losses ----------------
            # sq for this core's tiles, recomputed exactly from the f32 slice
            # (the program is SPMD-identical, so it cannot slice sqd at a
            # per-core offset; recomputing from fa32 is 3 cheap ops)
            fsq = sb.tile([128, TC * D], F32)
            f3s = fsq.rearrange("p (t d) -> p t d", d=D)
            nc.scalar.activation(f3s[:, :, :], fa3[:, :, 0:D], Act.Square)
            sq8 = sb.tile([128, TC], F32)
            nc.vector.tensor_reduce(sq8[:], f3s, axis=AxX, op=Alu.add)
            nc.vector.tensor_copy(fa3[:, :, D], sq8[:])

            lossrows = sb.tile([128, TC], F32)
            accpair = sb.tile([128, 2], F32)
            nc.gpsimd.memset(accpair[:, 1:2], 0.0)
            for j in range(TC):
                gP = ps.tile([128, W3], F32, tag="gps", bufs=2, name=f"g{j}")
                nc.tensor.matmul(gP[:], eohT[:, j * 128:(j + 1) * 128], rb[:],
                                 start=True, stop=True)
                pscr = sb.tile([128, W3], F32, tag="pscr", bufs=2,
                               name=f"p{j}")
                nc.vector.scalar_tensor_tensor(
                    pscr[:], gP[:], 0.0, fa3[:, j, 0:W3],
                    op0=Alu.bypass, op1=Alu.mult,
                    accum_out=lossrows[:, j:j + 1])
            relscr = sb.tile([128, TC], F32)
            nc.vector.tensor_scalar(relscr[:], lossrows[:], 0.0, None,
                                    op0=Alu.max, op1=Alu.add,
                                    accum_out=accpair[:, 0:1])
            nc.vector.tensor_tensor(accpair[0:16, 1:2], cnt, vmask[:],
                                    op=Alu.mult)
            nc.sync.dma_start(res, accpair[:])

    nc.compile()
    _CACHE["nc"] = nc
    return nc


def _make_in_maps(features, labels):
    feats = np.ascontiguousarray(np.asarray(features, dtype=np.float32))
    lab = np.ascontiguousarray(np.asarray(labels)).astype(np.float32)

    # (128, T, D) row-tile-major: row i = t*128 + p
    ftile = feats.reshape(T, 128, D).transpose(1, 0, 2)
    labtile = lab.reshape(T, 128).T                       # (128, T)

    fhb = np.zeros((128, T, W), ml_dtypes.bfloat16)
    fhb[:, :, 0:D] = ftile.astype(ml_dtypes.bfloat16)
    fhb[:, :, D + 1] = 1.0

    labio = np.zeros((128, T * 16 + 16), ml_dtypes.bfloat16)
    labio[:, 0:T * 16] = np.repeat(labtile, 16, axis=1).astype(
        ml_dtypes.bfloat16)
    labio[:, T * 16:] = np.arange(16, dtype=np.float32)[None, :].astype(
        ml_dtypes.bfloat16)

    smallf = np.zeros((16, 17), np.float32)
    smallf[:, 0] = np.arange(16, dtype=np.float32)
    smallf[:, 1:17] = 1.0

    shared = {
        "fhin": np.ascontiguousarray(fhb.reshape(128, T * W)),
        "labio": np.ascontiguousarray(labio),
        "smallf": smallf,
    }
    in_maps = []
    for c in range(NCORES):
        t0 = c * TC
        fa = np.zeros((128, TC, W3), np.float32)
        fa[:, :, 0:D] = ftile[:, t0:t0 + TC, :]
        fa[:, :, D + 1] = 1.0
        fa[:, :, D + 2] = 1.0
        lab16o = np.ascontiguousarray(np.broadcast_to(
            lab[t0 * 128:(t0 + TC) * 128], (16, TC * 128))).astype(
                ml_dtypes.bfloat16)
        m = dict(shared)
        m["fa32in"] = np.ascontiguousarray(fa.reshape(128, TC * W3))
        m["lab16o"] = lab16o
        in_maps.append(m)
    return in_maps


def kernel(features, labels):
    nc = _build()
    in_maps = _make_in_maps(features, labels)
    out = run_bass_kernel_spmd(nc, in_maps, core_ids=list(range(NCORES)))
    num = 0.0
    for r in out.results:
        num += float(r["res"][:, 0].sum())
    den = float(out.results[0]["res"][:, 1].sum())
    return np.float32(num / max(den, 1.0))
